# revision 21
# baseline (speedup 1.0000x reference)
"""CSNet-init patchify kernel for 8 TRN2 NeuronCores.

Computation (per sample, 1024x1024 image, 32x32 blocks):
  outcsy[c,h,w] = sum_{p,q} w_sample[c,p,q] * x[32h+p, 32w+q]        (strided conv)
  out[d,h,w]    = sum_c w_init[d,c] * outcsy[c,h,w]                  (1x1 conv)
  rec[32i+p,32j+q] = out[q*32+p, i, j]                               (block reshuffle)

Sharding: pure data parallel, batch 16 -> 2 samples per core, weights replicated.

Layout strategy (all DMAs are 128-partition x 4KB-contiguous):
  - load S_k[part=(p',ht), free=(w,q)] = rows 32*ht + 4k + p' of the image
  - DVE 32x32 block-transpose -> T_k[part=(p',q), free=(w,ht)]; (p', q) with
    p = 4k+p' is contraction-dim chunk k for the patchify matmul
  - matmul1 (accumulate over k): psum1[c', (h,w)] with rhs free enumerated (h,w)
  - matmul2 (accumulate over c-chunks): psum2[v=(p'',q), (j,i)] where the output
    channel grouping d = q*32 + 4m + p'' is chosen so that a second DVE 32x32
    block-transpose turns psum2 directly into rec rows [(p'',i), (j,q)]

Precision variants (rel-err tolerance is loose; fp32 storage throughout):
  - "f32":      fp32 matmuls (4 cyc/row, exact)
  - "f32r":     f32r matmuls (1 cyc/row); T rounded via gpsimd copy
  - "bf16-f32r": mm1 in bf16 (input cast during SWDGE DMA, no rounding pass),
                 mm2 in f32r (ACT psum->sbuf copy does the rounding)
"""

import numpy as np
import ml_dtypes
from contextlib import ExitStack

import concourse.bass as bass
import concourse.tile as tile
import concourse.mybir as mybir
from concourse import bacc, bass_utils

N_CORES = 8
SPC = 2  # samples per core (batch 16 / 8 cores)
F32 = mybir.dt.float32
F32R = mybir.dt.float32r
BF16 = mybir.dt.bfloat16

VARIANT = "bf16-f32r"

_NC_CACHE = {}


def _dtypes(variant):
    return {
        "f32": (F32, F32),
        "f32r": (F32R, F32R),
        "bf16-f32r": (BF16, F32R),
        "bf16": (BF16, BF16),
    }[variant]


def build_nc(variant=VARIANT, repeat=1):
    mdt1, mdt2 = _dtypes(variant)
    nc = bacc.Bacc("TRN2", target_bir_lowering=False, debug=False)

    x_d = nc.dram_tensor("x", [SPC, 1, 1024, 1024], F32, kind="ExternalInput").ap()
    ws_d = nc.dram_tensor("wsT", [128, 2048], mdt1, kind="ExternalInput").ap()
    wi_d = nc.dram_tensor("wiT", [128, 2048], mdt2, kind="ExternalInput").ap()
    rec_d = nc.dram_tensor("rec", [SPC, 1, 1024, 1024], F32, kind="ExternalOutput").ap()
    csy_d = nc.dram_tensor("outcsy", [SPC, 256, 32, 32], F32, kind="ExternalOutput").ap()
    out_d = nc.dram_tensor("out", [SPC, 1024, 32, 32], F32, kind="ExternalOutput").ap()

    with tile.TileContext(nc) as tc, ExitStack() as ctx:
        wp = ctx.enter_context(tc.tile_pool(name="w", bufs=1))
        sp = ctx.enter_context(tc.tile_pool(name="s", bufs=6))
        t0p = ctx.enter_context(tc.tile_pool(name="t0", bufs=6)) if mdt1 == F32R else None
        tp = ctx.enter_context(tc.tile_pool(name="t", bufs=17))
        o1p = ctx.enter_context(tc.tile_pool(name="o1", bufs=4))
        osp = ctx.enter_context(tc.tile_pool(name="osb", bufs=3))
        rsp = ctx.enter_context(tc.tile_pool(name="rsb", bufs=3))
        pp1 = ctx.enter_context(tc.tile_pool(name="pp1", bufs=4, space="PSUM"))
        pp2 = ctx.enter_context(tc.tile_pool(name="pp2", bufs=4, space="PSUM"))

        for _ in range(repeat):
            # phase A: weights, then all input loads + block transposes
            T = {}
            for n in range(SPC):
                if n == 0:
                    wsT = wp.tile([128, 2048], mdt1, tag="wsT", name="wsT")
                    nc.sync.dma_start(wsT[:], ws_d[:])
                else:
                    # wiT is first needed by mm2; keep it off the DMA head
                    wiT = wp.tile([128, 2048], mdt2, tag="wiT", name="wiT")
                    nc.sync.dma_start(wiT[:], wi_d[:])
                xv = x_d[n, 0].rearrange("(h g q) w -> g q h w", h=32, g=8, q=4)
                for k in range(8):
                    t = tp.tile([128, 1024], mdt1, tag="t", name="t")
                    if mdt1 == F32:
                        s = sp.tile([128, 1024], F32, tag="s", name="s")
                        nc.sync.dma_start(s[:], xv[k])
                        nc.vector.transpose(t[:], s[:])
                    elif mdt1 == BF16:
                        # SWDGE casts f32 -> bf16 during the load
                        s = sp.tile([128, 1024], BF16, tag="s", name="s")
                        nc.gpsimd.dma_start(s[:], xv[k])
                        nc.vector.transpose(t[:], s[:])
                    else:  # F32R: StreamTranspose is fp32-only; round afterwards
                        s = sp.tile([128, 1024], F32, tag="s", name="s")
                        nc.sync.dma_start(s[:], xv[k])
                        t0 = t0p.tile([128, 1024], F32, tag="t0", name="t0")
                        nc.vector.transpose(t0[:], s[:])
                        if k % 2 == 0:
                            nc.gpsimd.tensor_copy(t[:], t0[:])
                        else:
                            nc.scalar.copy(t[:], t0[:])
                    T[n, k] = t

            # phase B: patchify matmul, outcsy
            o1 = {}
            for n in range(SPC):
                csyv = csy_d[n].rearrange("c i j -> c (i j)")
                for cc in range(2):
                    o1[n, cc] = o1p.tile([128, 1024], mdt2, tag="o1", name="o1")
                for f in range(2):
                    ps1 = [pp1.tile([128, 512], F32, tag="ps1", name="ps1")
                           for _ in range(2)]
                    for k in range(8):
                        tr = T[n, k][:].rearrange("u (w h) -> u h w", w=32, h=32)
                        rhs = tr[:, 16 * f: 16 * (f + 1), :]
                        for cc in range(2):
                            lhs = wsT[:, k * 256 + cc * 128: k * 256 + cc * 128 + 128]
                            nc.tensor.matmul(
                                ps1[cc][:], lhs, rhs,
                                start=(k == 0), stop=(k == 7),
                            )
                    for cc in range(2):
                        nc.scalar.copy(
                            o1[n, cc][:, 512 * f: 512 * (f + 1)], ps1[cc][:]
                        )
                        # drain each outcsy half as soon as its copy lands
                        if mdt2 == BF16:
                            nc.gpsimd.dma_start(
                                csyv[cc * 128:(cc + 1) * 128, 512 * f: 512 * (f + 1)],
                                o1[n, cc][:, 512 * f: 512 * (f + 1)],
                            )
                        else:
                            nc.sync.dma_start(
                                csyv[cc * 128:(cc + 1) * 128, 512 * f: 512 * (f + 1)],
                                o1[n, cc][:, 512 * f: 512 * (f + 1)].bitcast(F32),
                            )

            # phase C: 1x1 conv, out + rec
            for n in range(SPC):
                outv = out_d[n].rearrange("(q g p) i j -> g p q (i j)", q=32, g=8, p=4)
                recv = rec_d[n, 0].rearrange("(i g q) w -> g q i w", i=32, g=8, q=4)
                for m in range(8):
                    osb = osp.tile([128, 1024], F32, tag="osb", name="osb")
                    rsb = rsp.tile([128, 1024], F32, tag="rsb", name="rsb")
                    for f in range(2):
                        ps2 = pp2.tile([128, 512], F32, tag="ps2", name="ps2")
                        for cc in range(2):
                            lhs = wiT[:, cc * 1024 + m * 128: cc * 1024 + m * 128 + 128]
                            rhs = o1[n, cc][:].rearrange(
                                "c (i j) -> c j i", i=32, j=32)[:, 16 * f: 16 * (f + 1), :]
                            nc.tensor.matmul(
                                ps2[:], lhs, rhs, start=(cc == 0), stop=(cc == 1)
                            )
                        src = ps2[:].rearrange("v (j i) -> v i j", j=16, i=32)
                        dst = osb[:].rearrange(
                            "v (i j) -> v i j", i=32, j=32)[:, :, 16 * f: 16 * (f + 1)]
                        nc.scalar.copy(dst, src)
                        nc.vector.transpose(rsb[:, 512 * f: 512 * (f + 1)], ps2[:])
                    nc.sync.dma_start(outv[m], osb[:])
                    nc.sync.dma_start(recv[m], rsb[:])

    nc.compile()
    return nc


def shuffle_weights(w_sample, w_init):
    """Host-side repack of the (tiny, replicated) conv weights into the SBUF
    layouts the kernel expects. wsT[u=(p',q), k*256+c] = w_sample[c,0,4k+p',q];
    wiT[c', cc*1024 + m*128 + v=(p'',q)] = w_init[q*32+4m+p'', cc*128+c']."""
    ws = np.ascontiguousarray(w_sample[:, 0])          # [256, 32, 32] (c, p, q)
    a = ws.reshape(256, 8, 4, 32)                      # c, k, p', q
    wsT = np.ascontiguousarray(a.transpose(2, 3, 1, 0)).reshape(128, 2048)
    wi = np.ascontiguousarray(w_init[:, :, 0, 0])      # [1024, 256] (d, c)
    b = wi.reshape(32, 8, 4, 256)                      # q, m, p'', c
    wiT = np.ascontiguousarray(b.transpose(3, 1, 2, 0))   # [256, 8, 4, 32]
    wiT = wiT.reshape(2, 128, 1024).transpose(1, 0, 2).reshape(128, 2048)
    return np.ascontiguousarray(wsT), np.ascontiguousarray(wiT)


def make_in_maps(x, w_sample, w_init, variant=VARIANT):
    mdt1, _ = _dtypes(variant)
    wsT, wiT = shuffle_weights(
        np.asarray(w_sample, dtype=np.float32), np.asarray(w_init, dtype=np.float32)
    )
    mdt2 = _dtypes(variant)[1]
    if mdt1 == BF16:
        wsT = wsT.astype(ml_dtypes.bfloat16)
    if mdt2 == BF16:
        wiT = wiT.astype(ml_dtypes.bfloat16)
    x = np.asarray(x, dtype=np.float32)
    return [
        {"x": np.ascontiguousarray(x[c * SPC:(c + 1) * SPC]), "wsT": wsT, "wiT": wiT}
        for c in range(N_CORES)
    ]


def _run(variant, x, w_sample, w_init):
    if variant not in _NC_CACHE:
        _NC_CACHE[variant] = build_nc(variant)
    nc = _NC_CACHE[variant]
    in_maps = make_in_maps(x, w_sample, w_init, variant)
    res = bass_utils.run_bass_kernel_spmd(
        nc, in_maps, core_ids=list(range(N_CORES)), trace=False
    )
    rec = np.concatenate([r["rec"] for r in res.results], axis=0)
    outcsy = np.concatenate([r["outcsy"] for r in res.results], axis=0)
    out = np.concatenate([r["out"] for r in res.results], axis=0)
    return rec, outcsy, out


def kernel(x, w_sample, w_init):
    try:
        return _run(VARIANT, x, w_sample, w_init)
    except Exception:
        # fall back to the exact-fp32 build (slower, bit-safe) if the
        # reduced-precision variant fails to compile/run in this environment
        if VARIANT == "f32":
            raise
        return _run("f32", x, w_sample, w_init)


# revision 29
# speedup vs baseline: 1.0603x; 1.0603x over previous
"""CSNet-init patchify kernel for 8 TRN2 NeuronCores.

Computation (per sample, 1024x1024 image, 32x32 blocks):
  outcsy[c,h,w] = sum_{p,q} w_sample[c,p,q] * x[32h+p, 32w+q]        (strided conv)
  out[d,h,w]    = sum_c w_init[d,c] * outcsy[c,h,w]                  (1x1 conv)
  rec[32i+p,32j+q] = out[q*32+p, i, j]                               (block reshuffle)

Sharding: pure data parallel, batch 16 -> 2 samples per core, weights replicated.

Layout strategy (all DMAs are 128-partition x 4KB-contiguous):
  - load S_k[part=(p',ht), free=(w,q)] = rows 32*ht + 4k + p' of the image
  - DVE 32x32 block-transpose -> T_k[part=(p',q), free=(w,ht)]; (p', q) with
    p = 4k+p' is contraction-dim chunk k for the patchify matmul
  - matmul1 (accumulate over k): psum1[c', (h,w)] with rhs free enumerated (h,w)
  - matmul2 (accumulate over c-chunks): psum2[v=(p'',q), (j,i)] where the output
    channel grouping d = q*32 + 4m + p'' is chosen so that a second DVE 32x32
    block-transpose turns psum2 directly into rec rows [(p'',i), (j,q)]

Precision variants (rel-err tolerance is loose; fp32 storage throughout):
  - "f32":      fp32 matmuls (4 cyc/row, exact)
  - "f32r":     f32r matmuls (1 cyc/row); T rounded via gpsimd copy
  - "bf16-f32r": mm1 in bf16 (input cast during SWDGE DMA, no rounding pass),
                 mm2 in f32r (ACT psum->sbuf copy does the rounding)
"""

import numpy as np
import ml_dtypes
from contextlib import ExitStack

import concourse.bass as bass
import concourse.tile as tile
import concourse.mybir as mybir
from concourse import bacc, bass_utils

N_CORES = 8
SPC = 2  # samples per core (batch 16 / 8 cores)
F32 = mybir.dt.float32
F32R = mybir.dt.float32r
BF16 = mybir.dt.bfloat16

VARIANT = "bf16-f32r"

_NC_CACHE = {}


def _dtypes(variant):
    return {
        "f32": (F32, F32),
        "f32r": (F32R, F32R),
        "bf16-f32r": (BF16, F32R),
        "bf16": (BF16, BF16),
    }[variant]


def build_nc(variant=VARIANT, repeat=1):
    mdt1, mdt2 = _dtypes(variant)
    nc = bacc.Bacc("TRN2", target_bir_lowering=False, debug=False)

    x_d = nc.dram_tensor("x", [SPC, 1, 1024, 1024], F32, kind="ExternalInput").ap()
    ws_d = nc.dram_tensor("wsT", [128, 2048], mdt1, kind="ExternalInput").ap()
    wi_d = nc.dram_tensor("wiT", [128, 2048], mdt2, kind="ExternalInput").ap()
    rec_d = nc.dram_tensor("rec", [SPC, 1, 1024, 1024], F32, kind="ExternalOutput").ap()
    csy_d = nc.dram_tensor("outcsy", [SPC, 256, 32, 32], F32, kind="ExternalOutput").ap()
    out_d = nc.dram_tensor("out", [SPC, 1024, 32, 32], F32, kind="ExternalOutput").ap()

    with tile.TileContext(nc) as tc, ExitStack() as ctx:
        wp = ctx.enter_context(tc.tile_pool(name="w", bufs=1))
        sp = ctx.enter_context(tc.tile_pool(name="s", bufs=12 if mdt1 == BF16 else 6))
        t0p = ctx.enter_context(tc.tile_pool(name="t0", bufs=6)) if mdt1 == F32R else None
        tp = ctx.enter_context(tc.tile_pool(name="t", bufs=17))
        o1p = ctx.enter_context(tc.tile_pool(name="o1", bufs=4))
        osp = ctx.enter_context(tc.tile_pool(name="osb", bufs=3))
        rsp = ctx.enter_context(tc.tile_pool(name="rsb", bufs=3))
        pp1 = ctx.enter_context(tc.tile_pool(name="pp1", bufs=4, space="PSUM"))
        pp2 = ctx.enter_context(tc.tile_pool(name="pp2", bufs=4, space="PSUM"))

        for _ in range(repeat):
            # phase A: weights, then all input loads + block transposes.
            # Sample 1's transposes are deferred (emitted after sample 0's
            # first rec block-transposes) so the DVE serves the rec path of
            # sample 0 first; extra s-pool slots let the loads run ahead.
            T = {}
            S1 = {}
            for n in range(SPC):
                if n == 0:
                    wsT = wp.tile([128, 2048], mdt1, tag="wsT", name="wsT")
                    nc.sync.dma_start(wsT[:], ws_d[:])
                else:
                    # wiT is first needed by mm2; keep it off the DMA head
                    wiT = wp.tile([128, 2048], mdt2, tag="wiT", name="wiT")
                    nc.sync.dma_start(wiT[:], wi_d[:])
                xv = x_d[n, 0].rearrange("(h g q) w -> g q h w", h=32, g=8, q=4)
                for k in range(8):
                    t = tp.tile([128, 1024], mdt1, tag="t", name="t")
                    if mdt1 == F32:
                        s = sp.tile([128, 1024], F32, tag="s", name="s")
                        nc.sync.dma_start(s[:], xv[k])
                        nc.vector.transpose(t[:], s[:])
                    elif mdt1 == BF16:
                        # SWDGE casts f32 -> bf16 during the load
                        s = sp.tile([128, 1024], BF16, tag="s", name="s")
                        nc.gpsimd.dma_start(s[:], xv[k])
                        if n == 0:
                            nc.vector.transpose(t[:], s[:])
                        else:
                            S1[k] = s
                    else:  # F32R: StreamTranspose is fp32-only; round afterwards
                        s = sp.tile([128, 1024], F32, tag="s", name="s")
                        nc.sync.dma_start(s[:], xv[k])
                        t0 = t0p.tile([128, 1024], F32, tag="t0", name="t0")
                        nc.vector.transpose(t0[:], s[:])
                        if k % 2 == 0:
                            nc.gpsimd.tensor_copy(t[:], t0[:])
                        else:
                            nc.scalar.copy(t[:], t0[:])
                    T[n, k] = t

            # phase B / C bodies (emission order = scheduler priority)
            o1 = {}

            def emit_B(n):
                csyv = csy_d[n].rearrange("c i j -> c (i j)")
                for cc in range(2):
                    o1[n, cc] = o1p.tile([128, 1024], mdt2, tag="o1", name="o1")
                for f in range(2):
                    ps1 = [pp1.tile([128, 512], F32, tag="ps1", name="ps1")
                           for _ in range(2)]
                    for k in range(8):
                        tr = T[n, k][:].rearrange("u (w h) -> u h w", w=32, h=32)
                        rhs = tr[:, 16 * f: 16 * (f + 1), :]
                        for cc in range(2):
                            lhs = wsT[:, k * 256 + cc * 128: k * 256 + cc * 128 + 128]
                            nc.tensor.matmul(
                                ps1[cc][:], lhs, rhs,
                                start=(k == 0), stop=(k == 7),
                            )
                    for cc in range(2):
                        nc.scalar.copy(
                            o1[n, cc][:, 512 * f: 512 * (f + 1)], ps1[cc][:]
                        )
                        # drain each outcsy half as soon as its copy lands
                        nc.sync.dma_start(
                            csyv[cc * 128:(cc + 1) * 128, 512 * f: 512 * (f + 1)],
                            o1[n, cc][:, 512 * f: 512 * (f + 1)].bitcast(F32),
                        )

            def emit_C(n, ms):
                outv = out_d[n].rearrange("(q g p) i j -> g p q (i j)", q=32, g=8, p=4)
                recv = rec_d[n, 0].rearrange("(i g q) w -> g q i w", i=32, g=8, q=4)
                for m in ms:
                    osb = osp.tile([128, 1024], F32, tag="osb", name="osb")
                    rsb = rsp.tile([128, 1024], F32, tag="rsb", name="rsb")
                    for f in range(2):
                        ps2 = pp2.tile([128, 512], F32, tag="ps2", name="ps2")
                        for cc in range(2):
                            lhs = wiT[:, cc * 1024 + m * 128: cc * 1024 + m * 128 + 128]
                            rhs = o1[n, cc][:].rearrange(
                                "c (i j) -> c j i", i=32, j=32)[:, 16 * f: 16 * (f + 1), :]
                            nc.tensor.matmul(
                                ps2[:], lhs, rhs, start=(cc == 0), stop=(cc == 1)
                            )
                        src = ps2[:].rearrange("v (j i) -> v i j", j=16, i=32)
                        dst = osb[:].rearrange(
                            "v (i j) -> v i j", i=32, j=32)[:, :, 16 * f: 16 * (f + 1)]
                        nc.scalar.copy(dst, src)
                        nc.vector.transpose(rsb[:, 512 * f: 512 * (f + 1)], ps2[:])
                    nc.sync.dma_start(outv[m], osb[:])
                    nc.sync.dma_start(recv[m], rsb[:])

            # sample 0's first out/rec tiles are emitted before sample 1's
            # patchify stage so their DMAs can fill the slot right after the
            # input loads drain
            emit_B(0)
            emit_C(0, range(0, 3))
            for k in sorted(S1):
                nc.vector.transpose(T[1, k][:], S1[k][:])
            S1.clear()
            emit_B(1)
            emit_C(0, range(3, 8))
            emit_C(1, range(0, 8))

    nc.compile()
    return nc


def shuffle_weights(w_sample, w_init):
    """Host-side repack of the (tiny, replicated) conv weights into the SBUF
    layouts the kernel expects. wsT[u=(p',q), k*256+c] = w_sample[c,0,4k+p',q];
    wiT[c', cc*1024 + m*128 + v=(p'',q)] = w_init[q*32+4m+p'', cc*128+c']."""
    ws = np.ascontiguousarray(w_sample[:, 0])          # [256, 32, 32] (c, p, q)
    a = ws.reshape(256, 8, 4, 32)                      # c, k, p', q
    wsT = np.ascontiguousarray(a.transpose(2, 3, 1, 0)).reshape(128, 2048)
    wi = np.ascontiguousarray(w_init[:, :, 0, 0])      # [1024, 256] (d, c)
    b = wi.reshape(32, 8, 4, 256)                      # q, m, p'', c
    wiT = np.ascontiguousarray(b.transpose(3, 1, 2, 0))   # [256, 8, 4, 32]
    wiT = wiT.reshape(2, 128, 1024).transpose(1, 0, 2).reshape(128, 2048)
    return np.ascontiguousarray(wsT), np.ascontiguousarray(wiT)


def make_in_maps(x, w_sample, w_init, variant=VARIANT):
    mdt1, _ = _dtypes(variant)
    wsT, wiT = shuffle_weights(
        np.asarray(w_sample, dtype=np.float32), np.asarray(w_init, dtype=np.float32)
    )
    mdt2 = _dtypes(variant)[1]
    if mdt1 == BF16:
        wsT = wsT.astype(ml_dtypes.bfloat16)
    if mdt2 == BF16:
        wiT = wiT.astype(ml_dtypes.bfloat16)
    x = np.asarray(x, dtype=np.float32)
    return [
        {"x": np.ascontiguousarray(x[c * SPC:(c + 1) * SPC]), "wsT": wsT, "wiT": wiT}
        for c in range(N_CORES)
    ]


def _run(variant, x, w_sample, w_init):
    if variant not in _NC_CACHE:
        _NC_CACHE[variant] = build_nc(variant)
    nc = _NC_CACHE[variant]
    in_maps = make_in_maps(x, w_sample, w_init, variant)
    res = bass_utils.run_bass_kernel_spmd(
        nc, in_maps, core_ids=list(range(N_CORES)), trace=False
    )
    rec = np.concatenate([r["rec"] for r in res.results], axis=0)
    outcsy = np.concatenate([r["outcsy"] for r in res.results], axis=0)
    out = np.concatenate([r["out"] for r in res.results], axis=0)
    return rec, outcsy, out


def kernel(x, w_sample, w_init):
    try:
        return _run(VARIANT, x, w_sample, w_init)
    except Exception:
        # fall back to the exact-fp32 build (slower, bit-safe) if the
        # reduced-precision variant fails to compile/run in this environment
        if VARIANT == "f32":
            raise
        return _run("f32", x, w_sample, w_init)
